# revision 58
# baseline (speedup 1.0000x reference)
"""Trainium2 Bass kernel for nn_Attention_326417514823.

Per-batch computation (B=8, N=2048, D=256), one batch per NeuronCore:
    S = Q @ K.T / sqrt(D)                  (N x N)
    S[q, :] = -1e9 where mask[q] == 0      (row masking by query index)
    A = softmax(S, axis=0)                 (normalize over q, per column k)
    A[q, :] = 0 where mask[q] == 0
    O = A @ V                              (N x D)

Key structural insight: masked queries produce exactly-zero output rows and
contribute nothing to the softmax normalizer c[k] = sum_q E[q,k].  The host
therefore COMPACTS the ~50% unmasked queries per batch (gather), pads them to
a fixed NQ, and the device only ever computes the active-query block.  All
device work (both matmuls, exp, DVE) shrinks by NQ/N with no precision loss.
The host scatters the compacted output rows back (zeros elsewhere).

Device layout per core (transposed so the softmax reduction runs along the
free axis and neither matmul needs an on-chip transpose):
    ST[k, q] = KT.T @ QT        fp8e4m3 DoubleRow matmuls (0.5 cycles/row,
                                D=256 contracted in one 2x128 pass) with
                                3-term error compensation:
                                  S = K8.Q8 + dK8.Q8 + K8.dQ8
                                (dX8 = fp8 of the fp8-rounding residual; the
                                dropped dK.dQ term is ~0.1% on E — more
                                accurate than bf16 scores, at 75% of the PE
                                cost).  Host packs [K8|dK8] planes in one
                                tensor.
    E[k, q]  = exp(ST/16)       one ACT instruction per 128-row k-block over
                                the full NQ (PSUM read spanning 3 banks)
    c[k]     = sum_q E - npad   (DVE 2x-mode reduce over the bf16 E row;
                                 padded q columns are zero => exp(0)=1 each;
                                 npad is passed per-core as a tiny input)
    W[k, :]  = V[k, :] * (1/c[k])                   (bf16)
    OT[d, q] = sum_k W[k,d] * E[k,q]  (bf16, PSUM accumulation over k-blocks;
                                 fp8 DoubleRow here fails the 2e-2 gate:
                                 measured 5.1e-2 uncompensated)
Host transposes/scatters OT back to O.

PSUM (8 banks): 2 resident banks hold OT accumulators for q-chunk 0 (both
d-halves) for the whole phase-1 loop (interleaved matmul-2, LAG k-blocks
behind the softmax pipeline); score tiles [128, NQ] f32 double-buffer in
2 x 3 banks.  K-block 0 scores its first 1024 columns into the still-idle
resident banks instead, so its exp runs as three early slices and the whole
serial ACT exp chain (the pacing resource together with the PE) starts ~1 us
sooner.  Phase 2 re-uses the psS pool's own buffers for the remaining
q-chunk accumulation chains (a fresh pool would barrier on every psS
reader), ordered so only the final small store's fixed pipeline (copy,
HWDGE ring prep ~630 ns, DGE delay ~650 ns, transfer, 900 ns DMA semaphore,
engine drains) trails the last matmul.

DMA choreography: emissions ordered by first consumption, kt pre-packed
into partition-major slabs (2 KB descriptors; sub-512 B descriptors pay a
2x DMA latency penalty), compute-engine rings kept free of mid-loop DMA
configs (a config queued behind the shared HWDGE backlog would block exp
issue), and the PE kept busy by warmup matmuls during the fill (the cost
model needs ~3 us of cumulative PE busy to reach the 2.4 GHz p-state).

Measured: 34.2 us (TimelineSim cost model; bf16 baseline of this same
structure was 40.9 us, the pre-session baseline 67.9 us), rel_of_scale
5.4e-3 on hardware.
"""

import numpy as np
import ml_dtypes

B, N, D = 8, 2048, 256
NCORES = 8
P = 128          # partitions
KB = N // P      # 16 k-blocks
DT = D // P      # 2 d-halves
NQ_DEFAULT = 1072  # padded compacted query count (max active for seed(0): 1070)
LAG = 3          # k-blocks of slack before interleaved matmul-2 consumes E/W
WARMUP = 29      # dummy matmuls covering the PE p-state ramp during DMA fill
                 # (cost model: full clock after ~3 us of cumulative PE busy)

bf = ml_dtypes.bfloat16

_cached = {}


def _chunks(nq):
    """q-chunks of at most 512 (PSUM bank of fp32)."""
    out = []
    off = 0
    while off < nq:
        w = min(512, nq - off)
        out.append((off, w))
        off += w
    return out


def _build(nq):
    import concourse.bacc as bacc
    import concourse.mybir as mybir
    import concourse.tile as tile

    f32 = mybir.dt.float32
    bf16 = mybir.dt.bfloat16
    fp8 = mybir.dt.float8e4
    EXP = mybir.ActivationFunctionType.Exp
    SUB = mybir.AluOpType.subtract
    ADD = mybir.AluOpType.add
    DR = mybir.MatmulPerfMode.DoubleRow
    CH = _chunks(nq)
    # score chunks for DoubleRow matmul-1: moving free = 2*w <= 512
    SCH = []
    for off, w in CH:
        for o2 in range(off, off + w, 256):
            SCH.append((o2, min(256, off + w - o2)))

    nc = bacc.Bacc()
    # kt is pre-packed on the host into partition-major 512-k-column group
    # slabs [group, p, (plane, d-half, 512)] so each group loads as 128
    # contiguous 2 KB descriptors (small descriptors pay a 2x DMA latency
    # penalty).  qt packs the fp8 value plane and its residual plane; row
    # (t*128+p) of the [256, *] host matrix lands at partition p, free index
    # (t, :) — exactly the (d_lo, d_hi) pairing DoubleRow wants.
    ktd = nc.dram_tensor("kt", [4, P, 4 * 512], fp8, kind="ExternalInput")
    qtd = nc.dram_tensor("qt", [2, D, nq], fp8, kind="ExternalInput")
    vd = nc.dram_tensor("v", [N, D], bf16, kind="ExternalInput")
    padd = nc.dram_tensor("padc", [1, 1], f32, kind="ExternalInput")
    otd = nc.dram_tensor("ot", [D, nq], bf16, kind="ExternalOutput")

    def dview8(dram, c0, w):
        """[2, 256, w] dram slice as [128, 2(plane), 2(d-half), w]."""
        return dram[:, :, c0:c0 + w].rearrange("pl (t p) w -> p pl t w", p=P)

    def dview(dram, c0, w):
        """[256, w] dram slice as [128, 2, w] (d-halves on the free axis)."""
        return dram[:, c0:c0 + w].rearrange("(t p) w -> p t w", p=P)

    with tile.TileContext(nc) as tc:
        with (
            tc.tile_pool(name="const", bufs=1) as constp,
            tc.tile_pool(name="epool", bufs=1) as epool,
            tc.tile_pool(name="wpool", bufs=1) as wpool,
            tc.tile_pool(name="vpool", bufs=4) as vpool,
            tc.tile_pool(name="cpool", bufs=4) as cpool,
            tc.tile_pool(name="outp", bufs=3) as outp,
            # q-chunk-0 OT accumulators live for the whole phase 1 (2 banks)
            tc.tile_pool(name="psA", bufs=1, space="PSUM") as psA,
        ):
            # resident accumulators: q-chunk 0, both d-halves
            accA = [psA.tile([P, 512], f32, name=f"accA{dh}")
                    for dh in range(DT)]

            # Warm the PE (p-state ramp) while the fill DMAs run; the garbage
            # lands in accA and is cleared by the first start=True matmul.
            zs = constp.tile([P, P], bf16, name="zs")
            nc.gpsimd.memset(zs[:], 0.0)
            for _ in range(WARMUP):
                nc.tensor.matmul(accA[0][:, 0:P], zs[:], zs[:],
                                 start=True, stop=True)

            # Every input DMA is emitted up-front (kt groups on the ACT ring,
            # the rest on the SP ring): mid-loop dma_start configs would
            # stall a compute sequencer behind the shared HWDGE queue.
            kt_g = [constp.tile([P, 2, DT, 512], fp8, name=f"ktg{j}")
                    for j in range(4)]
            # qt: q-chunk 0 first (gates the first matmul), remainder second
            qt_c0 = constp.tile([P, 2, DT, 512], fp8, name="qtc0")
            qt_cr = constp.tile([P, 2, DT, nq - 512], fp8, name="qtcr")
            nc.sync.dma_start(qt_c0[:], dview8(qtd, 0, 512))
            nc.scalar.dma_start(kt_g[0][:], ktd[0])
            nc.sync.dma_start(qt_cr[:], dview8(qtd, 512, nq - 512))
            nc.scalar.dma_start(kt_g[1][:], ktd[1])
            padb = constp.tile([P, 1], f32, name="padb")
            nc.sync.dma_start(padb[:], padd[0:1, :].partition_broadcast(P))

            def qt_mv(pl, off, w):
                if off + w <= 512:
                    return qt_c0[:, pl, :, off:off + w]
                return qt_cr[:, pl, :, off - 512:off - 512 + w]

            def kt_st(kb, pl):
                return kt_g[kb // 4][:, pl, :, (kb % 4) * P:(kb % 4 + 1) * P]

            # V loads batched 4 k-blocks per DMA: [128, 4(sub), 256(d)]
            v_grps = []
            for g in range(KB // 4):
                v_g = vpool.tile([P, 4, D], bf16, name=f"v_g{g}")
                src = vd[g * 4 * P:(g + 1) * 4 * P, :].rearrange(
                    "(s p) d -> p s d", p=P)
                if g == 0:
                    nc.sync.dma_start(v_g[:], src)
                v_grps.append(v_g)
            # groups 2-3 on the SP ring: a config queued on the ACT ring
            # would block exp0's issue behind the shared HWDGE backlog
            for j in range(2, 4):
                nc.sync.dma_start(kt_g[j][:], ktd[j])
            for g in range(1, KB // 4):
                src = vd[g * 4 * P:(g + 1) * 4 * P, :].rearrange(
                    "(s p) d -> p s d", p=P)
                nc.sync.dma_start(v_grps[g][:], src)

            def v_slice(kb):
                return v_grps[kb // 4][:, kb % 4, :]
            # Preload the Exp activation table during the fill as well.
            ewarm = cpool.tile([P, 1], f32, name="ewarm")
            nc.scalar.activation(ewarm[:], zs[:, 0:1], EXP, scale=0.0)

            e_all = [None] * KB
            w_all = [None] * KB

            def mm2(acc, kb, dh, off, w):
                nc.tensor.matmul(
                    acc[:, 0:w],
                    w_all[kb][:, dh * P:(dh + 1) * P],
                    e_all[kb][:, off:off + w],
                    start=(kb == 0),
                    stop=(kb == KB - 1),
                )

            with tc.tile_pool(name="psS", bufs=2, space="PSUM") as psS:
                for kb in range(KB):
                    # matmul-1: 3-term compensated fp8 DoubleRow scores for
                    # this k-block, all q chunks, into one [128, nq] f32 PSUM
                    # tile spanning 3 banks.  k-block 0 instead scores its
                    # first 1024 columns into the (still idle) resident accA
                    # banks as separate tiles, so its exp runs as three
                    # slices, the first starting ~1 us earlier — this pulls
                    # the whole serial ACT exp chain forward.
                    st = psS.tile([P, nq], f32, name="st")

                    def target(off, w):
                        if kb == 0 and off + w <= 512:
                            return accA[0][:, off:off + w]
                        if kb == 0 and off + w <= 1024:
                            return accA[1][:, off - 512:off - 512 + w]
                        return st[:, off:off + w]

                    for off, w in SCH:
                        for i, (pst, pmv) in enumerate(
                                [(0, 0), (1, 0), (0, 1)]):
                            # within one accA bank the second 256-chunk must
                            # NOT re-raise start (it would mark the whole
                            # 2 KB zero-region pending and wipe the first
                            # chunk); its bytes are still pending from the
                            # first chunk's start and auto-zero on first use
                            first = (i == 0) and not (
                                kb == 0 and off in (256, 768))
                            nc.tensor.matmul(
                                target(off, w),
                                kt_st(kb, pst),
                                qt_mv(pmv, off, w),
                                start=first,
                                stop=(i == 2),
                                perf_mode=DR,
                                skip_group_check=(kb == 0 and off < 1024),
                            )
                    e_kb = epool.tile([P, nq], bf16, name=f"e{kb}")
                    if kb == 0:
                        nc.scalar.activation(e_kb[:, 0:512], accA[0][:],
                                             EXP, scale=1.0 / 16.0)
                        nc.scalar.activation(e_kb[:, 512:1024], accA[1][:],
                                             EXP, scale=1.0 / 16.0)
                        nc.scalar.activation(e_kb[:, 1024:nq],
                                             st[:, 1024:nq], EXP,
                                             scale=1.0 / 16.0)
                    else:
                        nc.scalar.activation(e_kb[:], st[:], EXP,
                                             scale=1.0 / 16.0)
                    # c-sum rides a 4x-mode tensor_scalar identity multiply
                    # (tensor_reduce / scalar_tensor_tensor have no DVE fast
                    # modes in the cost model)
                    c_acc = cpool.tile([P, 1], f32, name="c_acc")
                    nc.vector.tensor_scalar(
                        e_kb[:], e_kb[:], 1.0, 0.0,
                        mybir.AluOpType.mult, mybir.AluOpType.add,
                        accum_out=c_acc[:])
                    # c = c_acc - npad  (each padded q column contributes
                    # exp(0) = 1 to the sum)
                    rc = cpool.tile([P, 1], f32, name="rc")
                    nc.vector.tensor_tensor(c_acc[:], c_acc[:], padb[:], SUB)
                    nc.vector.reciprocal(rc[:], c_acc[:])
                    w_kb = wpool.tile([P, D], bf16, name=f"w{kb}")
                    nc.vector.tensor_scalar_mul(w_kb[:], v_slice(kb), rc[:])
                    e_all[kb] = e_kb
                    w_all[kb] = w_kb

                    # interleaved matmul-2 on q-chunk 0, LAG k-blocks behind
                    # (the last LAG k-blocks are finished inside phase 2,
                    # after the first chain, so the PE never waits on w15)
                    if kb >= LAG:
                        for dh in range(DT):
                            mm2(accA[dh], kb - LAG, dh, 0, 512)

                # Phase 2 (still inside the psS pool: a fresh pool here
                # would open with a barrier on ALL psS readers, stalling the
                # first chain on exp15; psS's own rotation hands out the
                # buffer freed by exp14 instead).
                def chain(dh, off, w, engine):
                    acc = psS.tile([P, nq], f32, name="st")
                    for kb in range(KB):
                        mm2(acc, kb, dh, off, w)
                    o_sb = outp.tile([P, w], bf16, name="o_ch")
                    if engine == "act":
                        nc.scalar.copy(o_sb[:], acc[:, 0:w])
                    else:
                        nc.vector.tensor_copy(o_sb[:], acc[:, 0:w])
                    nc.sync.dma_start(
                        otd[dh * P:(dh + 1) * P, off:off + w], o_sb[:])

                # d-half-1 pieces over [512, nq): progressively narrower so
                # every fixed store cost (ring prep, DGE delay, sem) except
                # the last hides under later chains, ending on a small store
                rest = nq - 512
                tailp, r = [], rest
                if r > 560:
                    tailp, r = [304, 128], r - 432
                elif r > 256:
                    tailp, r = [128], r - 128
                while r > 0:
                    w = min(512, r)
                    tailp.insert(0, w)
                    r -= w
                # first chain runs while the softmax pipeline drains (only
                # its k-block-15 matmul waits on w15)
                chain(0, 512, min(256, rest), "act")
                # finish the resident q-chunk-0 accumulators and store them
                # (both d-halves share one staging tile and one store DMA)
                for j in range(KB - LAG, KB):
                    for dh in range(DT):
                        mm2(accA[dh], j, dh, 0, 512)
                o_qc0 = outp.tile([P, DT, 512], bf16, name="o_qc0")
                nc.scalar.copy(o_qc0[:, 0, :], accA[0][:])
                nc.vector.tensor_copy(o_qc0[:, 1, :], accA[1][:])
                nc.sync.dma_start(dview(otd, 0, 512), o_qc0[:])
                off = 512 + min(256, rest)
                while off < nq:
                    w = min(512, nq - off)
                    chain(0, off, w, "dve")
                    off += w
                off = 512
                for i, w in enumerate(tailp):
                    chain(1, off, w, "dve" if i % 2 == 0 else "act")
                    off += w

    nc.compile()
    return nc


def _get_nc(nq=NQ_DEFAULT):
    if nq not in _cached:
        _cached[nq] = _build(nq)
    return _cached[nq]


def kernel(key, query, value, mask):
    from concourse.bass_utils import run_bass_kernel_spmd

    key = np.asarray(key, dtype=np.float32)
    query = np.asarray(query, dtype=np.float32)
    value = np.asarray(value, dtype=np.float32)
    mask = np.asarray(mask)

    idxs = [np.nonzero(mask[b, 0])[0] for b in range(B)]
    n_acts = [len(ix) for ix in idxs]
    nq = NQ_DEFAULT
    if max(n_acts) > nq:
        # robustness fallback for inputs denser than the compiled default;
        # 1536 is the PSUM limit (3-bank score tiles), beyond it batches are
        # finished on the host (impossible for Bernoulli(0.5) masks)
        nq = min(1536, ((max(n_acts) + 255) // 256) * 256)
    host_batches = [b for b in range(B) if n_acts[b] > nq]
    nc = _get_nc(nq)

    f8 = ml_dtypes.float8_e4m3

    def pack8(x):
        """[rows, cols] f32 -> [2, rows, cols] fp8 (value, residual)."""
        hi = x.astype(f8)
        lo = (x - hi.astype(np.float32)).astype(f8)
        return np.stack([hi, lo])

    def pack8_kt(x):
        """[256, 2048] f32 K.T -> [4, 128, 2048] fp8 partition-major group
        slabs: [group, p, (plane, d-half, 512 k-columns)]."""
        pl = pack8(x)                                   # [2, 256, 2048]
        pl = pl.reshape(2, 2, P, 4, 512)                # (pl, t, p, g, w)
        return np.ascontiguousarray(
            pl.transpose(3, 2, 0, 1, 4).reshape(4, P, 4 * 512))

    in_maps = []
    for b in range(B):
        na = min(n_acts[b], nq)
        qt = np.zeros((D, nq), dtype=np.float32)
        if na:
            qt[:, :na] = query[b][idxs[b][:na]].T
        in_maps.append({
            "kt": pack8_kt(np.ascontiguousarray(key[b].T)),
            "qt": pack8(qt),
            "v": value[b].astype(bf),
            "padc": np.full((1, 1), float(nq - na), np.float32),
        })
    res = None
    for attempt in range(4):
        try:
            res = run_bass_kernel_spmd(nc, in_maps, core_ids=list(range(NCORES)))
            break
        except Exception:
            # Transient "accelerator device unrecoverable" states wedge the
            # PJRT client but not the device: tear down the backend and retry.
            if attempt == 3:
                raise
            import time
            time.sleep(10 * (attempt + 1))
            try:
                import jax.extend.backend as _jb
                _jb.clear_backends()
                import jax
                jax.clear_caches()
            except Exception:
                pass
    out = np.zeros((B, N, D), np.float32)
    for b in range(B):
        if b in host_batches:
            # exact host path for adversarially dense masks
            ix = idxs[b]
            s = query[b][ix] @ key[b].T / np.float32(np.sqrt(D))
            e = np.exp(s - s.max(axis=0, keepdims=True))
            out[b][ix] = (e / e.sum(axis=0, keepdims=True)) @ value[b]
            continue
        na = n_acts[b]
        if na:
            out[b][idxs[b]] = res.results[b]["ot"][:, :na].T.astype(np.float32)
    return out
